# revision 3
# baseline (speedup 1.0000x reference)
"""Trainium2 Bass kernel for nn_BiStackedLSTMOne.

Model (per reference):
  forward stack: frames 30..61 (32 steps) -> LSTM(512->256) -> LSTM(256->256)
  reverse stack: frames 63,62,61 (3 steps) -> LSTM(512->256) -> LSTM(256->256)
  out = concat(hF, hR) @ W3.T + b3        # (B, 10)

Distribution: data-parallel over batch. 2048 rows -> 8 NeuronCores x 256.

Device layout: "chunk-major, feature-on-partition". A logical (F, B) tensor
with F = nchunks*128 lives in SBUF as (128, nchunks, B). Gates are computed
transposed - gates'[j, b] - so the hidden state h is produced directly in the
layout the next matmul consumes. Nothing is transposed on device; the host
pre-transposes xs and pre-packs the weights.

v2 (scalar-engine diet): the baseline spent ~300us on 702 ACTIVATEs (each
(N+352)/1.2 ns, i.e. ~290ns fixed overhead at N=256). Now each LSTM cell
step uses two 2-bank PSUM tiles holding gate blocks [f,f,i,i] and [o,o,g,g]
and only 4 ACTIVATEs: sigmoid(N=1024), sigmoid(N=512), tanh(N=512),
tanh_c(N=512). The per-block bias (which forced per-block ACTs before) is
instead accumulated into PSUM by 4 tiny K=2 "indicator" matmuls that open
each bank's accumulation group (row-tiled to positions 0/32/64/96 so they
run concurrently on the PE sub-arrays). All matmul operands are bf16 so
LDWEIGHTS uses fast-weight-load and hides fully under the N=256 matmul;
PSUM accumulation stays fp32, cell state stays fp32.
"""

import os
import sys

sys.path.insert(0, "/opt/trn_rl_repo")
if "/root/.axon_site" not in sys.path:
    sys.path.insert(0, "/root/.axon_site")

import numpy as np
import ml_dtypes

import concourse.bacc as bacc
import concourse.bass as bass
import concourse.mybir as mybir
import concourse.tile as tile
from concourse.bass_utils import run_bass_kernel_spmd

F32 = mybir.dt.float32
BF16 = mybir.dt.bfloat16
AF = mybir.ActivationFunctionType

NCORES = 8
BC = 256          # batch rows per core
TF = 32           # forward steps (frames 30..61)
TR = 3            # reverse steps (frames 63,62,61)
NT = TF + TR      # x time slots shipped to device
HID = 256
NBLK = 8          # 4H / 128 gate blocks
# gate blocks after host permutation: f (0,1) i (2,3) o (4,5) g (6,7)
# torch order is i (0,1) f (2,3) g (4,5) o (6,7)
GATE_PERM = [2, 3, 0, 1, 6, 7, 4, 5]
LAYER_IDX = {"f0": 0, "f1": 1, "r0": 2, "r1": 3}

LAST_RESULTS = {"exec_time_ns": None}


def _install_ntff_hook():
    """Recreate the missing antenv.axon_hooks shim so trace=True works."""
    import types

    try:
        import antenv
    except ImportError:
        return
    if "antenv.axon_hooks" in sys.modules:
        return
    mod = types.ModuleType("antenv.axon_hooks")
    mod._hook = None
    mod.set_axon_ntff_profile_hook = lambda h: setattr(mod, "_hook", h)
    mod.get_axon_ntff_profile_hook = lambda: mod._hook
    sys.modules["antenv.axon_hooks"] = mod
    antenv.axon_hooks = mod
    try:
        from trn_agent_boot.trn_boot import _ntff_profile_via_ctypes

        hook = _ntff_profile_via_ctypes("/opt/axon/libaxon_pjrt.so")
        if hook is not None:
            mod.set_axon_ntff_profile_hook(hook)
    except Exception:
        pass


def build_nc():
    nc = bacc.Bacc(None, target_bir_lowering=False, debug=False)

    x_d = nc.declare_dram_parameter("x", [NT, 128, 4, BC], BF16, isOutput=False)
    w_d = {}
    for name, kc in [("wih_f0", 4), ("whh_f0", 2), ("wih_f1", 2), ("whh_f1", 2),
                     ("wih_r0", 4), ("whh_r0", 2), ("wih_r1", 2),
                     ("whh_r1", 2)]:
        w_d[name] = nc.declare_dram_parameter(name, [128, kc, NBLK, 128], BF16,
                                              isOutput=False)
    # bias as matmul lhsT rows: partition 32q+r holds the bias vector of gate
    # block 2q+r (bank q of the step's two psum tiles), per layer
    bmm_d = nc.declare_dram_parameter("biasmm", [128, 4, 128], BF16,
                                      isOutput=False)
    # indicator rhs rows: partition 32q+0 = [1]*256+[0]*256, 32q+1 = inverse
    ind_d = nc.declare_dram_parameter("ind", [128, 512], BF16, isOutput=False)
    w3_d = nc.declare_dram_parameter("w3", [128, 4, 16], BF16, isOutput=False)
    b3_d = nc.declare_dram_parameter("b3", [16, 1], F32, isOutput=False)
    out_d = nc.declare_dram_parameter("out", [16, BC], F32, isOutput=True)

    with tile.TileContext(nc) as tc:
        with (
            tc.tile_pool(name="wpool", bufs=1) as wpool,
            tc.tile_pool(name="xpool", bufs=6) as xpool,
            tc.tile_pool(name="pspool", bufs=4, space="PSUM") as pspool,
            tc.tile_pool(name="apool", bufs=6) as apool,
            tc.tile_pool(name="spool", bufs=3) as spool,
            tc.tile_pool(name="hpool", bufs=4) as hpool,
            tc.tile_pool(name="cpool", bufs=1) as cpool,
            tc.tile_pool(name="opool", bufs=1) as opool,
        ):
            # preload the sigmoid/tanh ACT table set while DMAs run
            warm = opool.tile([1, 2], F32, tag="warm")
            nc.vector.memset(warm[:], 0.0)
            nc.scalar.activation(warm[:, 0:1], warm[:, 0:1], AF.Sigmoid)
            # keep the PE's HAM clock warm during the startup DMA window
            wz = opool.tile([128, BC], F32, tag="warm_z")
            nc.gpsimd.memset(wz[:], 0.0)
            wzr = opool.tile([128, BC], BF16, tag="warm_zr")
            nc.gpsimd.tensor_copy(wzr[:], wz[:])
            wps = pspool.tile([128, 4, BC], F32, tag="ps")
            for _ in range(32):
                nc.tensor.matmul(wps[:, 0, :], wzr[:, :128], wzr[:],
                                 start=True, stop=True)

            # ---- x streaming ----
            xs = {}

            def load_x(t):
                xt = xpool.tile([128, 4, BC], BF16, tag="x", name=f"x{t}")
                if t < 2:
                    for kc in range(4):
                        nc.sync.dma_start(xt[:, kc, :], x_d.ap()[t, :, kc, :])
                else:
                    nc.sync.dma_start(xt[:], x_d.ap()[t])
                xs[t] = xt

            # ---- one-time: weights + bias rows + indicator rows ----
            w = {}

            def load_w(name, kcs=None):
                dram = w_d[name]
                nkc = dram.shape[1]
                tiles = w.setdefault(name, [None] * nkc)
                for kc in (range(nkc) if kcs is None else kcs):
                    t = wpool.tile([128, NBLK, 128], BF16, tag=f"{name}_{kc}",
                                   name=f"{name}_{kc}")
                    nc.sync.dma_start(t[:], dram.ap()[:, kc])
                    tiles[kc] = t

            bmm = wpool.tile([128, 4, 128], BF16, tag="biasmm")
            ind = wpool.tile([128, 512], BF16, tag="ind")

            # interleave the loads step-0 needs so the first MMs start ASAP
            nc.sync.dma_start(bmm[:], bmm_d.ap())
            nc.sync.dma_start(ind[:], ind_d.ap())
            load_w("wih_f0", [0])
            load_x(0)
            load_w("wih_f0", [1])
            load_x(1)
            load_w("wih_f0", [2, 3])
            load_w("wih_f1")
            load_w("whh_f0")
            load_w("whh_f1")
            load_x(2)
            load_x(3)

            def load_rest(stage):
                if stage == 0:
                    load_w("wih_r0", [0, 1])
                elif stage == 1:
                    load_w("wih_r0", [2, 3])
                    load_w("whh_r0")
                elif stage == 2:
                    load_w("wih_r1")
                    load_w("whh_r1")
                elif stage == 3:
                    w3 = wpool.tile([128, 4, 16], BF16, tag="w3")
                    nc.sync.dma_start(w3[:], w3_d.ap())
                    b3 = wpool.tile([16, 1], F32, tag="b3")
                    nc.sync.dma_start(b3[:], b3_d.ap())
                    wb3.extend([w3, b3])

            wb3 = []

            def lstm_step(lname, x_in, kc_in, first, c_t, h_prev,
                          rec_first=False):
                """One LSTM cell step in transposed layout.

                x_in: list of (128, BC) chunk APs. h_prev: (128, 2, BC) bf16
                tile or None. c_t: persistent (128, 2, BC) f32 tile.
                Gate blocks: ps1 = [f,f,i,i], ps2 = [o,o,g,g].
                Returns h as a fresh (128, 2, BC) bf16 tile.
                """
                wih = w[f"wih_{lname}"]
                whh = w[f"whh_{lname}"]
                lidx = LAYER_IDX[lname]
                ps1 = pspool.tile([128, 4, BC], F32, tag="ps")
                ps2 = pspool.tile([128, 4, BC], F32, tag="ps")
                banks = [ps1[:, 0:2, :], ps1[:, 2:4, :],
                         ps2[:, 0:2, :], ps2[:, 2:4, :]]
                # bias: open each bank's accumulation group with a K=2
                # indicator matmul; row-tiled so the 4 run concurrently
                for q in range(4):
                    nc.tensor.matmul(
                        banks[q], bmm[32 * q:32 * q + 2, lidx, :],
                        ind[32 * q:32 * q + 2, :],
                        start=True, stop=False, tile_position=(32 * q, 0),
                        skip_group_check=True,
                    )
                n_in_group = kc_in + (0 if first else 2)
                for m in range(NBLK):
                    ps = ps1 if m < 4 else ps2
                    po = ps[:, m % 4, :]
                    gi = 0
                    inp = [(wih[kc], x_in[kc]) for kc in range(kc_in)]
                    rec = ([] if first else
                           [(whh[kc], h_prev[:, kc, :]) for kc in (0, 1)])
                    # L0: input first (hoistable ahead of h_prev).
                    # L1: rec first (h_prev-only dep fills the h0 wait).
                    ops = rec + inp if rec_first else inp + rec
                    for wt, rhs_ap in ops:
                        gi += 1
                        nc.tensor.matmul(
                            po, wt[:, m, :], rhs_ap,
                            start=False,
                            stop=(m % 2 == 1 and gi == n_in_group),
                            skip_group_check=True,
                        )
                a1 = apool.tile([128, 4, BC], F32, tag="a1")
                a2 = apool.tile([128, 4, BC], F32, tag="a2")
                nc.scalar.activation(a1[:], ps1[:], AF.Sigmoid)
                nc.scalar.activation(a2[:, 2:4, :], ps2[:, 2:4, :], AF.Tanh)
                s_f, s_i = a1[:, 0:2, :], a1[:, 2:4, :]
                s_o, t_g = a2[:, 0:2, :], a2[:, 2:4, :]
                if first:
                    nc.vector.tensor_mul(c_t[:], s_i, t_g)
                else:
                    nc.vector.tensor_mul(c_t[:], s_f, c_t[:])
                    m1 = spool.tile([128, 2, BC], F32, tag="m1")
                    nc.gpsimd.tensor_mul(m1[:], s_i, t_g)
                    nc.vector.tensor_add(c_t[:], c_t[:], m1[:])
                tc_ = spool.tile([128, 2, BC], F32, tag="tc")
                nc.scalar.activation(tc_[:], c_t[:], AF.Tanh)
                nc.scalar.activation(a2[:, 0:2, :], ps2[:, 0:2, :], AF.Sigmoid)
                h = hpool.tile([128, 2, BC], BF16, tag=f"h_{lname}",
                               name=f"h_{lname}")
                nc.vector.tensor_mul(h[:], s_o, tc_[:])
                return h

            # ---- forward stack, reverse stack interleaved as PE filler ----
            c = {ln: cpool.tile([128, 2, BC], F32, tag=f"c_{ln}",
                                name=f"c_{ln}")
                 for ln in ["f0", "f1", "r0", "r1"]}
            R0_AT = {5: 0, 15: 1, 28: 2}      # fwd step -> rev-layer0 step
            R1_AT = {7: 0, 17: 1, 30: 2}      # fwd step -> rev-layer1 step
            h0 = h1 = None
            r0 = r1 = None
            for t in range(TF):
                xa = [xs[t][:, kc, :] for kc in range(4)]
                h0 = lstm_step("f0", xa, 4, t == 0, c["f0"], h0)
                del xs[t]
                if t in R0_AT:
                    r = R0_AT[t]
                    xr = [xs[TF + r][:, kc, :] for kc in range(4)]
                    r0 = lstm_step("r0", xr, 4, r == 0, c["r0"], r0)
                    del xs[TF + r]
                if t in R1_AT:
                    r = R1_AT[t]
                    xr1 = [r0[:, kc, :] for kc in (0, 1)]
                    r1 = lstm_step("r1", xr1, 2, r == 0, c["r1"], r1,
                                   rec_first=True)
                xh0 = [h0[:, kc, :] for kc in (0, 1)]
                h1 = lstm_step("f1", xh0, 2, t == 0, c["f1"], h1,
                               rec_first=True)
                if t in (2, 3, 4, 5):
                    load_rest(t - 2)
                # prefetch: fwd t+4, plus the rev slot two steps early
                if t + 4 < TF:
                    load_x(t + 4)
                if t + 2 in R0_AT:
                    load_x(TF + R0_AT[t + 2])
            hF = h1
            hR = r1

            # ---- classifier: out[n,b] = sum_k W3[n,k] latent[k,b] + b3 ----
            ps = pspool.tile([128, 4, BC], F32, tag="ps")
            po = ps[:16, 0, :]
            w3, b3 = wb3
            nc.tensor.matmul(po, w3[:, 2, :], hR[:, 0, :], start=True,
                             stop=False)
            nc.tensor.matmul(po, w3[:, 3, :], hR[:, 1, :], start=False,
                             stop=False)
            nc.tensor.matmul(po, w3[:, 0, :], hF[:, 0, :], start=False,
                             stop=False)
            nc.tensor.matmul(po, w3[:, 1, :], hF[:, 1, :], start=False,
                             stop=True)
            ot = opool.tile([16, BC], F32, tag="out")
            nc.scalar.add(ot[:], po, b3[:])
            nc.sync.dma_start(out_d.ap(), ot[:])

    nc.compile()
    return nc


def _bf16(x):
    return np.ascontiguousarray(x, dtype=np.float32).astype(ml_dtypes.bfloat16)


def _pack_weights(Wih, Whh):
    """Pack into lhsT chunk layout: W.T tiles (128, KC, 8, 128), bf16."""
    fourH, D = Wih.shape
    kc_i, kc_h = D // 128, Whh.shape[1] // 128
    wih = np.ascontiguousarray(
        Wih.reshape(NBLK, 128, kc_i, 128)[GATE_PERM].transpose(3, 2, 0, 1))
    whh = np.ascontiguousarray(
        Whh.reshape(NBLK, 128, kc_h, 128)[GATE_PERM].transpose(3, 2, 0, 1))
    return _bf16(wih), _bf16(whh)


_NC_CACHE = {}


def kernel(xs, Wih_f0, Whh_f0, bih_f0, bhh_f0, Wih_f1, Whh_f1, bih_f1, bhh_f1,
           Wih_r0, Whh_r0, bih_r0, bhh_r0, Wih_r1, Whh_r1, bih_r1, bhh_r1,
           W3, b3):
    if os.environ.get("BASS_TRACE"):
        _install_ntff_hook()

    if "nc" not in _NC_CACHE:
        _NC_CACHE["nc"] = build_nc()
    nc = _NC_CACHE["nc"]

    B = xs.shape[0]
    assert B == NCORES * BC

    # frames used: 30..61 forward, then 63,62,61 reversed order
    frames = list(range(62 - TF, 62)) + [63, 62, 61]
    # (B, NT, 512) -> (NT, 512, B)
    xsel = np.ascontiguousarray(
        xs[:, frames, :].transpose(1, 2, 0)).astype(np.float32)

    common = {}
    biasmm = np.zeros((128, 4, 128), np.float32)
    for lname, (Wih, Whh, bih, bhh) in {
        "f0": (Wih_f0, Whh_f0, bih_f0, bhh_f0),
        "f1": (Wih_f1, Whh_f1, bih_f1, bhh_f1),
        "r0": (Wih_r0, Whh_r0, bih_r0, bhh_r0),
        "r1": (Wih_r1, Whh_r1, bih_r1, bhh_r1),
    }.items():
        wih, whh = _pack_weights(np.asarray(Wih), np.asarray(Whh))
        common[f"wih_{lname}"] = wih
        common[f"whh_{lname}"] = whh
        b_all = (np.asarray(bih) + np.asarray(bhh)).reshape(NBLK, 128)[GATE_PERM]
        li = LAYER_IDX[lname]
        for q in range(4):
            biasmm[32 * q + 0, li, :] = b_all[2 * q + 0]
            biasmm[32 * q + 1, li, :] = b_all[2 * q + 1]
    common["biasmm"] = _bf16(biasmm)
    ind = np.zeros((128, 512), np.float32)
    for q in range(4):
        ind[32 * q + 0, :256] = 1.0
        ind[32 * q + 1, 256:] = 1.0
    common["ind"] = _bf16(ind)

    W3 = np.asarray(W3, dtype=np.float32)          # (10, 512)
    w3p = np.zeros((128, 4, 16), np.float32)
    w3p[:, :, :10] = W3.reshape(10, 4, 128).transpose(2, 1, 0)
    common["w3"] = _bf16(w3p)
    b3p = np.zeros((16, 1), np.float32)
    b3p[:10, 0] = np.asarray(b3, dtype=np.float32)
    common["b3"] = b3p

    in_maps = []
    for core in range(NCORES):
        m = dict(common)
        xc = xsel[:, :, core * BC:(core + 1) * BC].reshape(NT, 4, 128, BC)
        m["x"] = _bf16(np.ascontiguousarray(xc.transpose(0, 2, 1, 3)))
        in_maps.append(m)

    res = run_bass_kernel_spmd(nc, in_maps, list(range(NCORES)))
    LAST_RESULTS["exec_time_ns"] = res.exec_time_ns
    LAST_RESULTS["raw"] = res

    out = np.concatenate(
        [res.results[c]["out"][:10, :].T for c in range(NCORES)], axis=0)
    return np.ascontiguousarray(out.astype(np.float32))


# revision 6
# speedup vs baseline: 1.1855x; 1.1855x over previous
"""Trainium2 Bass kernel for nn_BiStackedLSTMOne.

Model (per reference):
  forward stack: frames 30..61 (32 steps) -> LSTM(512->256) -> LSTM(256->256)
  reverse stack: frames 63,62,61 (3 steps) -> LSTM(512->256) -> LSTM(256->256)
  out = concat(hF, hR) @ W3.T + b3        # (B, 10)

Distribution: data-parallel over batch. 2048 rows -> 8 NeuronCores x 256.

Device layout: "chunk-major, feature-on-partition". A logical (F, B) tensor
with F = nchunks*128 lives in SBUF as (128, nchunks, B). Gates are computed
transposed - gates'[j, b] - so the hidden state h is produced directly in the
layout the next matmul consumes. Nothing is transposed on device; the host
pre-transposes xs and pre-packs the weights.

v2 (scalar-engine diet): the baseline spent ~300us on 702 ACTIVATEs (each
(N+352)/1.2 ns, i.e. ~290ns fixed overhead at N=256). Now each LSTM cell
step uses two 2-bank PSUM tiles holding gate blocks [f,f,i,i] and [o,o,g,g]
and only 4 ACTIVATEs: sigmoid(N=1024), sigmoid(N=512), tanh(N=512),
tanh_c(N=512). The per-block bias (which forced per-block ACTs before) is
instead accumulated into PSUM by 4 tiny K=2 "indicator" matmuls that open
each bank's accumulation group (row-tiled to positions 0/32/64/96 so they
run concurrently on the PE sub-arrays). All matmul operands are bf16 so
LDWEIGHTS uses fast-weight-load and hides fully under the N=256 matmul;
PSUM accumulation stays fp32, cell state stays fp32.
"""

import os
import sys

sys.path.insert(0, "/opt/trn_rl_repo")
if "/root/.axon_site" not in sys.path:
    sys.path.insert(0, "/root/.axon_site")

import numpy as np
import ml_dtypes

import concourse.bacc as bacc
import concourse.bass as bass
import concourse.mybir as mybir
import concourse.tile as tile
from concourse.bass_utils import run_bass_kernel_spmd

F32 = mybir.dt.float32
BF16 = mybir.dt.bfloat16
AF = mybir.ActivationFunctionType

NCORES = 8
BC = 256          # batch rows per core
TF = 32           # forward steps (frames 30..61)
TR = 3            # reverse steps (frames 63,62,61)
NT = TF + TR      # x time slots shipped to device
HID = 256
NBLK = 8          # 4H / 128 gate blocks
# gate blocks after host permutation: f (0,1) i (2,3) g (4,5) o (6,7)
# torch order is i (0,1) f (2,3) g (4,5) o (6,7)
GATE_PERM = [2, 3, 0, 1, 4, 5, 6, 7]
LAYER_IDX = {"f0": 0, "f1": 1, "r0": 2, "r1": 3}

LAST_RESULTS = {"exec_time_ns": None}


def _install_ntff_hook():
    """Recreate the missing antenv.axon_hooks shim so trace=True works."""
    import types

    try:
        import antenv
    except ImportError:
        return
    if "antenv.axon_hooks" in sys.modules:
        return
    mod = types.ModuleType("antenv.axon_hooks")
    mod._hook = None
    mod.set_axon_ntff_profile_hook = lambda h: setattr(mod, "_hook", h)
    mod.get_axon_ntff_profile_hook = lambda: mod._hook
    sys.modules["antenv.axon_hooks"] = mod
    antenv.axon_hooks = mod
    try:
        from trn_agent_boot.trn_boot import _ntff_profile_via_ctypes

        hook = _ntff_profile_via_ctypes("/opt/axon/libaxon_pjrt.so")
        if hook is not None:
            mod.set_axon_ntff_profile_hook(hook)
    except Exception:
        pass


def build_nc():
    nc = bacc.Bacc(None, target_bir_lowering=False, debug=False)

    x_d = nc.declare_dram_parameter("x", [NT, 128, 4, BC], BF16, isOutput=False)
    w_d = {}
    for name, kc in [("wih_f0", 4), ("whh_f0", 2), ("wih_f1", 2), ("whh_f1", 2),
                     ("wih_r0", 4), ("whh_r0", 2), ("wih_r1", 2),
                     ("whh_r1", 2)]:
        w_d[name] = nc.declare_dram_parameter(name, [128, kc, NBLK, 128], BF16,
                                              isOutput=False)
    # bias as matmul lhsT rows: partition 32q+r holds the bias vector of gate
    # block 2q+r (bank q of the step's two psum tiles), per layer
    bmm_d = nc.declare_dram_parameter("biasmm", [128, 4, 128], BF16,
                                      isOutput=False)
    # indicator rhs rows: partition 32q+0 = [1]*256+[0]*256, 32q+1 = inverse
    ind_d = nc.declare_dram_parameter("ind", [128, 512], BF16, isOutput=False)
    w3_d = nc.declare_dram_parameter("w3", [128, 4, 16], BF16, isOutput=False)
    b3_d = nc.declare_dram_parameter("b3", [16, 1], F32, isOutput=False)
    out_d = nc.declare_dram_parameter("out", [16, BC], F32, isOutput=True)

    with tile.TileContext(nc) as tc:
        with (
            tc.tile_pool(name="wpool", bufs=1) as wpool,
            tc.tile_pool(name="xpool", bufs=6) as xpool,
            tc.tile_pool(name="pspool", bufs=4, space="PSUM") as pspool,
            tc.tile_pool(name="apool", bufs=6) as apool,
            tc.tile_pool(name="spool", bufs=3) as spool,
            tc.tile_pool(name="hpool", bufs=4) as hpool,
            tc.tile_pool(name="cpool", bufs=1) as cpool,
            tc.tile_pool(name="opool", bufs=1) as opool,
        ):
            # preload the sigmoid/tanh ACT table set while DMAs run
            warm = opool.tile([1, 2], F32, tag="warm")
            nc.vector.memset(warm[:], 0.0)
            nc.scalar.activation(warm[:, 0:1], warm[:, 0:1], AF.Sigmoid)
            # keep the PE's HAM clock warm during the startup DMA window
            wz = opool.tile([128, BC], F32, tag="warm_z")
            nc.gpsimd.memset(wz[:], 0.0)
            wzr = opool.tile([128, BC], BF16, tag="warm_zr")
            nc.gpsimd.tensor_copy(wzr[:], wz[:])
            wps = pspool.tile([128, 4, BC], F32, tag="ps")
            for _ in range(32):
                nc.tensor.matmul(wps[:, 0, :], wzr[:, :128], wzr[:],
                                 start=True, stop=True)

            # ---- x streaming ----
            xs = {}

            def load_x(t):
                xt = xpool.tile([128, 4, BC], BF16, tag="x", name=f"x{t}")
                if t < 2:
                    for kc in range(4):
                        nc.sync.dma_start(xt[:, kc, :], x_d.ap()[t, :, kc, :])
                else:
                    nc.sync.dma_start(xt[:], x_d.ap()[t])
                xs[t] = xt

            # ---- one-time: weights + bias rows + indicator rows ----
            w = {}

            def load_w(name, kcs=None):
                dram = w_d[name]
                nkc = dram.shape[1]
                tiles = w.setdefault(name, [None] * nkc)
                for kc in (range(nkc) if kcs is None else kcs):
                    t = wpool.tile([128, NBLK, 128], BF16, tag=f"{name}_{kc}",
                                   name=f"{name}_{kc}")
                    nc.sync.dma_start(t[:], dram.ap()[:, kc])
                    tiles[kc] = t

            bmm = wpool.tile([128, 4, 128], BF16, tag="biasmm")
            ind = wpool.tile([128, 512], BF16, tag="ind")

            # interleave the loads step-0 needs so the first MMs start ASAP
            nc.sync.dma_start(bmm[:], bmm_d.ap())
            nc.sync.dma_start(ind[:], ind_d.ap())
            load_w("wih_f0", [0])
            load_x(0)
            load_w("wih_f0", [1])
            load_x(1)
            load_w("wih_f0", [2, 3])
            load_w("wih_f1")
            load_w("whh_f0")
            load_w("whh_f1")
            load_x(2)
            load_x(3)

            def load_rest(stage):
                if stage == 0:
                    load_w("wih_r0", [0, 1])
                elif stage == 1:
                    load_w("wih_r0", [2, 3])
                    load_w("whh_r0")
                elif stage == 2:
                    load_w("wih_r1")
                    load_w("whh_r1")
                elif stage == 3:
                    w3 = wpool.tile([128, 4, 16], BF16, tag="w3")
                    nc.sync.dma_start(w3[:], w3_d.ap())
                    b3 = wpool.tile([16, 1], F32, tag="b3")
                    nc.sync.dma_start(b3[:], b3_d.ap())
                    wb3.extend([w3, b3])

            wb3 = []

            def step_A(lname, a_ops, nb_ops):
                """Phase A of one LSTM cell step: allocate the two gate psum
                tiles, open each bank's accumulation group with a K=2
                indicator bias matmul (row-tiled so the 4 run concurrently),
                then emit the matmuls whose operands are already available
                (x projections for L0, own-h recurrent for L1).

                a_ops: list of (weight_tile, rhs_ap) available now.
                nb_ops: how many more ops phase B will add per block (0 if
                this phase closes the accumulation groups).
                Gate blocks: ps1 = [f,f,i,i], ps2 = [g,g,o,o].
                """
                lidx = LAYER_IDX[lname]
                ps1 = pspool.tile([128, 4, BC], F32, tag="ps")
                ps2 = pspool.tile([128, 4, BC], F32, tag="ps")
                banks = [ps1[:, 0:2, :], ps1[:, 2:4, :],
                         ps2[:, 0:2, :], ps2[:, 2:4, :]]
                for q in range(4):
                    nc.tensor.matmul(
                        banks[q], bmm[32 * q:32 * q + 2, lidx, :],
                        ind[32 * q:32 * q + 2, :],
                        start=True, stop=False, tile_position=(32 * q, 0),
                        skip_group_check=True,
                    )
                for m in range(NBLK):
                    ps = ps1 if m < 4 else ps2
                    po = ps[:, m % 4, :]
                    for gi, (wt, rhs_ap) in enumerate(a_ops):
                        nc.tensor.matmul(
                            po, wt[:, m, :], rhs_ap,
                            start=False,
                            stop=(nb_ops == 0 and m % 2 == 1
                                  and gi == len(a_ops) - 1),
                            skip_group_check=True,
                        )
                return ps1, ps2

            def step_B(lname, ps12, b_ops, first, c_t):
                """Phase B: the h-dependent matmuls plus the whole
                elementwise tail. Returns h (128, 2, BC) bf16."""
                ps1, ps2 = ps12
                for m in range(NBLK):
                    ps = ps1 if m < 4 else ps2
                    po = ps[:, m % 4, :]
                    for gi, (wt, rhs_ap) in enumerate(b_ops):
                        nc.tensor.matmul(
                            po, wt[:, m, :], rhs_ap,
                            start=False,
                            stop=(m % 2 == 1 and gi == len(b_ops) - 1),
                            skip_group_check=True,
                        )
                a1 = apool.tile([128, 4, BC], F32, tag="a1")
                a2 = apool.tile([128, 4, BC], F32, tag="a2")
                nc.scalar.activation(a1[:], ps1[:], AF.Sigmoid)
                nc.scalar.activation(a2[:, 0:2, :], ps2[:, 0:2, :], AF.Tanh)
                s_f, s_i = a1[:, 0:2, :], a1[:, 2:4, :]
                t_g, s_o = a2[:, 0:2, :], a2[:, 2:4, :]
                if first:
                    nc.vector.tensor_mul(c_t[:], s_i, t_g)
                else:
                    nc.vector.tensor_mul(c_t[:], s_f, c_t[:])
                    m1 = spool.tile([128, 2, BC], F32, tag="m1")
                    nc.vector.tensor_mul(m1[:], s_i, t_g)
                    nc.vector.tensor_add(c_t[:], c_t[:], m1[:])
                tc_ = spool.tile([128, 2, BC], F32, tag="tc")
                nc.scalar.activation(tc_[:], c_t[:], AF.Tanh)
                # deferred: o-gate sigmoid lands right after tanh(c) so h
                # exits the ScalarE queue as early as possible
                nc.scalar.activation(a2[:, 2:4, :], ps2[:, 2:4, :], AF.Sigmoid)
                h = hpool.tile([128, 2, BC], BF16, tag=f"h_{lname}",
                               name=f"h_{lname}")
                nc.vector.tensor_mul(h[:], s_o, tc_[:])
                return h

            # ---- forward stack, reverse stack interleaved as PE filler ----
            # Emission order per step t:
            #   f0_t.B (rec on h0_{t-1} + tail -> h0_t)   [x-proj came at t-1]
            #   r0/r1 full steps (their deps are >= 2 steps old)
            #   f1_t.A (bias + rec on h1_{t-1})
            #   f0_{t+1}.A (bias + x projections)  <- PE filler while h0_t
            #   f1_t.B (input matmuls on h0_t + tail -> h1_t)
            c = {ln: cpool.tile([128, 2, BC], F32, tag=f"c_{ln}",
                                name=f"c_{ln}")
                 for ln in ["f0", "f1", "r0", "r1"]}
            R0_AT = {5: 0, 15: 1, 28: 2}      # fwd step -> rev-layer0 step
            R1_AT = {7: 0, 17: 1, 30: 2}      # fwd step -> rev-layer1 step
            h0 = h1 = None
            r0 = r1 = None

            def xops(t):
                return [(w["wih_f0"][kc], xs[t][:, kc, :]) for kc in range(4)]

            def hops(wname, h):
                return [(w[wname][kc], h[:, kc, :]) for kc in (0, 1)]

            f0_ps = step_A("f0", xops(0), 0)
            for t in range(TF):
                h0 = step_B("f0", f0_ps, [] if t == 0 else
                            hops("whh_f0", h0), t == 0, c["f0"])
                del xs[t]
                if t in R0_AT:
                    r = R0_AT[t]
                    r0_ops = [(w["wih_r0"][kc], xs[TF + r][:, kc, :])
                              for kc in range(4)]
                    nb = 0 if r == 0 else 2
                    r0_ps = step_A("r0", r0_ops, nb)
                    r0 = step_B("r0", r0_ps, [] if r == 0 else
                                hops("whh_r0", r0), r == 0, c["r0"])
                    del xs[TF + r]
                if t in R1_AT:
                    r = R1_AT[t]
                    r1_ps = step_A("r1", [] if r == 0 else
                                   hops("whh_r1", r1), 2)
                    r1 = step_B("r1", r1_ps, hops("wih_r1", r0), r == 0,
                                c["r1"])
                f1_ps = step_A("f1", [] if t == 0 else
                               hops("whh_f1", h1), 2)
                if t + 1 < TF:
                    f0_ps = step_A("f0", xops(t + 1), 2)
                h1 = step_B("f1", f1_ps, hops("wih_f1", h0), t == 0, c["f1"])
                if t in (2, 3, 4, 5):
                    load_rest(t - 2)
                # prefetch: fwd t+4, plus the rev slot two steps early
                if t + 4 < TF:
                    load_x(t + 4)
                if t + 2 in R0_AT:
                    load_x(TF + R0_AT[t + 2])
            hF = h1
            hR = r1

            # ---- classifier: out[n,b] = sum_k W3[n,k] latent[k,b] + b3 ----
            ps = pspool.tile([128, 4, BC], F32, tag="ps")
            po = ps[:16, 0, :]
            w3, b3 = wb3
            nc.tensor.matmul(po, w3[:, 2, :], hR[:, 0, :], start=True,
                             stop=False)
            nc.tensor.matmul(po, w3[:, 3, :], hR[:, 1, :], start=False,
                             stop=False)
            nc.tensor.matmul(po, w3[:, 0, :], hF[:, 0, :], start=False,
                             stop=False)
            nc.tensor.matmul(po, w3[:, 1, :], hF[:, 1, :], start=False,
                             stop=True)
            ot = opool.tile([16, BC], F32, tag="out")
            nc.scalar.add(ot[:], po, b3[:])
            nc.sync.dma_start(out_d.ap(), ot[:])

    nc.compile()
    return nc


def _bf16(x):
    return np.ascontiguousarray(x, dtype=np.float32).astype(ml_dtypes.bfloat16)


def _pack_weights(Wih, Whh):
    """Pack into lhsT chunk layout: W.T tiles (128, KC, 8, 128), bf16."""
    fourH, D = Wih.shape
    kc_i, kc_h = D // 128, Whh.shape[1] // 128
    wih = np.ascontiguousarray(
        Wih.reshape(NBLK, 128, kc_i, 128)[GATE_PERM].transpose(3, 2, 0, 1))
    whh = np.ascontiguousarray(
        Whh.reshape(NBLK, 128, kc_h, 128)[GATE_PERM].transpose(3, 2, 0, 1))
    return _bf16(wih), _bf16(whh)


_NC_CACHE = {}


def kernel(xs, Wih_f0, Whh_f0, bih_f0, bhh_f0, Wih_f1, Whh_f1, bih_f1, bhh_f1,
           Wih_r0, Whh_r0, bih_r0, bhh_r0, Wih_r1, Whh_r1, bih_r1, bhh_r1,
           W3, b3):
    if os.environ.get("BASS_TRACE"):
        _install_ntff_hook()

    if "nc" not in _NC_CACHE:
        _NC_CACHE["nc"] = build_nc()
    nc = _NC_CACHE["nc"]

    B = xs.shape[0]
    assert B == NCORES * BC

    # frames used: 30..61 forward, then 63,62,61 reversed order
    frames = list(range(62 - TF, 62)) + [63, 62, 61]
    # (B, NT, 512) -> (NT, 512, B)
    xsel = np.ascontiguousarray(
        xs[:, frames, :].transpose(1, 2, 0)).astype(np.float32)

    common = {}
    biasmm = np.zeros((128, 4, 128), np.float32)
    for lname, (Wih, Whh, bih, bhh) in {
        "f0": (Wih_f0, Whh_f0, bih_f0, bhh_f0),
        "f1": (Wih_f1, Whh_f1, bih_f1, bhh_f1),
        "r0": (Wih_r0, Whh_r0, bih_r0, bhh_r0),
        "r1": (Wih_r1, Whh_r1, bih_r1, bhh_r1),
    }.items():
        wih, whh = _pack_weights(np.asarray(Wih), np.asarray(Whh))
        common[f"wih_{lname}"] = wih
        common[f"whh_{lname}"] = whh
        b_all = (np.asarray(bih) + np.asarray(bhh)).reshape(NBLK, 128)[GATE_PERM]
        li = LAYER_IDX[lname]
        for q in range(4):
            biasmm[32 * q + 0, li, :] = b_all[2 * q + 0]
            biasmm[32 * q + 1, li, :] = b_all[2 * q + 1]
    common["biasmm"] = _bf16(biasmm)
    ind = np.zeros((128, 512), np.float32)
    for q in range(4):
        ind[32 * q + 0, :256] = 1.0
        ind[32 * q + 1, 256:] = 1.0
    common["ind"] = _bf16(ind)

    W3 = np.asarray(W3, dtype=np.float32)          # (10, 512)
    w3p = np.zeros((128, 4, 16), np.float32)
    w3p[:, :, :10] = W3.reshape(10, 4, 128).transpose(2, 1, 0)
    common["w3"] = _bf16(w3p)
    b3p = np.zeros((16, 1), np.float32)
    b3p[:10, 0] = np.asarray(b3, dtype=np.float32)
    common["b3"] = b3p

    in_maps = []
    for core in range(NCORES):
        m = dict(common)
        xc = xsel[:, :, core * BC:(core + 1) * BC].reshape(NT, 4, 128, BC)
        m["x"] = _bf16(np.ascontiguousarray(xc.transpose(0, 2, 1, 3)))
        in_maps.append(m)

    res = run_bass_kernel_spmd(nc, in_maps, list(range(NCORES)))
    LAST_RESULTS["exec_time_ns"] = res.exec_time_ns
    LAST_RESULTS["raw"] = res

    out = np.concatenate(
        [res.results[c]["out"][:10, :].T for c in range(NCORES)], axis=0)
    return np.ascontiguousarray(out.astype(np.float32))
